# revision 76
# baseline (speedup 1.0000x reference)
"""Trainium2 8-core kernel for RMSNorm -> QKV -> RoPE -> causal SDPA -> out-proj.

Sharding: core c = b*4 + g handles batch b (of 2) and heads 4g..4g+3 (of 16).
Each core computes a partial out-projection [dim, tokens]; the host sums the
4 head-group partials per batch and adds b_o.

Key layout/scheduling choices (cost model: matmul time = out-free-size rows,
vector/scalar op time = max free size):
- RMSNorm r is folded into BOTH Q and K rope tables (cosr/sinr = cos/sin * r),
  so softmax exp needs no per-key scale; V gets r via a token-major
  tensor_scalar. r itself comes from a DVE square + kc-tree-sum + gpsimd
  partition_all_reduce (no PE time), plus 16 one-row ones-matmuls for the
  token-major r_tok.
- rotate-half is 4 partition-permuting SBUF->SBUF DMAs per projection block
  (no PE perm matmul, no PSUM).
- Scores are computed key-major [128 keys, q] with exact causal starts.
- AV is computed TRANSPOSED: lhsT = exp tile [128 keys, 128 q], rhs = V
  [128 keys, 65] -> avt [q, d] at 65 rows/matmul (the PE streams only the
  65-wide rhs). Column 64 accumulates the softmax denominator (V's 65th
  column is ones); normalization is a per-partition reciprocal + one
  stride-0-broadcast multiply. Each q-tile's key-block accumulation is
  emitted as CONSECUTIVE matmuls: on real HW a PSUM bank supports only one
  open accumulation group at a time (interleaved-region accumulation that
  the interpreter accepts silently corrupts on silicon).
- av (token-major) -> feature-major for the out-projection via PE identity
  transposes (the XBAR dma_start_transpose's blocked-3D mode does not match
  interpreter semantics on HW).
- V projection and the second half of the QKV projections are emitted as
  "filler" work inside the attention kb loops so the PE stays busy while the
  Activation engine (the attention-era bottleneck) streams exp; AV groups
  lag the exp stream by one key block and sweep finalization rides the same
  deferred queue past head boundaries.
"""

import os

import numpy as np
import ml_dtypes

BF16 = ml_dtypes.bfloat16

DIM = 1024
HEADS = 16
DIM_HEAD = 64
T = 2048  # tokens per batch
B = 2
HPC = 4  # heads per core
F = HPC * DIM_HEAD  # 256 per-core head width
KC = DIM // 128  # 8 contraction chunks

_NC_CACHE = {}


def _build_nc():
    import concourse.bacc as bacc
    import concourse.mybir as mybir
    import concourse.tile as tile
    from concourse import bass_isa
    from contextlib import ExitStack

    f32 = mybir.dt.float32
    bf16 = mybir.dt.bfloat16
    nc = bacc.Bacc()

    xT = nc.declare_dram_parameter("xT", [DIM, T], bf16, isOutput=False)
    wq = nc.declare_dram_parameter("wq", [DIM, F], bf16, isOutput=False)
    wk = nc.declare_dram_parameter("wk", [DIM, F], bf16, isOutput=False)
    wv = nc.declare_dram_parameter("wv", [DIM, F], bf16, isOutput=False)
    wo = nc.declare_dram_parameter("wo", [F, DIM], bf16, isOutput=False)
    cosT = nc.declare_dram_parameter("cosT", [128, T], bf16, isOutput=False)
    sinT = nc.declare_dram_parameter("sinT", [128, T], bf16, isOutput=False)
    masks = nc.declare_dram_parameter("masks", [128, 128], bf16, isOutput=False)
    ident = nc.declare_dram_parameter("ident", [128, 128], bf16, isOutput=False)
    out = nc.declare_dram_parameter("out", [DIM, T], bf16, isOutput=True)
    debug = bool(int(os.environ.get("KERNEL_DEBUG", "0")))
    if debug:
        d_rbc = nc.declare_dram_parameter("d_rbc", [128, T], mybir_dt_f32 := __import__("concourse.mybir", fromlist=["dt"]).dt.float32, isOutput=True)
        d_qk = nc.declare_dram_parameter("d_qk", [128, 4 * T], bf16, isOutput=True)
        d_qk2 = nc.declare_dram_parameter("d_qk2", [128, 4 * T], bf16, isOutput=True)
        d_v = nc.declare_dram_parameter("d_v", [128, 16 * HPC * 65], bf16, isOutput=True)
        d_avtok = nc.declare_dram_parameter("d_avtok", [128, HPC * 16 * 64], bf16, isOutput=True)
        d_avfm = nc.declare_dram_parameter("d_avfm", [128, 2 * T], bf16, isOutput=True)

    Exp = mybir.ActivationFunctionType.Exp
    Sqrt = mybir.ActivationFunctionType.Sqrt
    mult = mybir.AluOpType.mult
    add = mybir.AluOpType.add

    nc_ = nc

    def stt_mul(out, a, b):
        return nc_.vector.tensor_mul(out, a, b)

    def stt_add(out, a, b):
        return nc_.vector.tensor_tensor(out, a, b, add)

    with ExitStack() as ctx:
        tc = ctx.enter_context(tile.TileContext(nc))
        consts = ctx.enter_context(tc.tile_pool(name="consts", bufs=1))
        persist = ctx.enter_context(tc.tile_pool(name="persist", bufs=1))
        work = ctx.enter_context(tc.tile_pool(name="work", bufs=2))
        vecs = ctx.enter_context(tc.tile_pool(name="vecs", bufs=2))

        # ---- constants / inputs ----
        wq_sb = consts.tile([128, KC, F], bf16, tag="wq")
        wk_sb = consts.tile([128, KC, F], bf16, tag="wk")
        wv_sb = consts.tile([128, KC, F], bf16, tag="wv")
        wo_sb = consts.tile([128, 2, DIM], bf16, tag="wo")
        cos_sb = consts.tile([128, T], bf16, tag="cos")
        sin_sb = consts.tile([128, T], bf16, tag="sin")
        mask_sb = consts.tile([128, 128], bf16, tag="mask")
        ones_col = consts.tile([128, 1], bf16, tag="onesc")
        id_sb = consts.tile([128, 128], bf16, tag="ident")

        xT_sb = persist.tile([128, KC, T], bf16, tag="xT")
        qk_sb = persist.tile([128, 4, T], bf16, tag="qk")  # raw projections
        qk2_sb = persist.tile([128, 4, T], bf16, tag="qk2")  # roped projections
        v_sb = persist.tile([128, 16, HPC, 65], bf16, tag="v")
        cosr_sb = persist.tile([128, T], bf16, tag="cosr")
        sinr_sb = persist.tile([128, T], bf16, tag="sinr")
        sq_tok = persist.tile([128, 16], f32, tag="sqtok")
        r_tok = persist.tile([128, 16], f32, tag="rtok")
        av_tok = persist.tile([128, HPC, 16, DIM_HEAD], bf16, tag="avtok")
        av_fm = persist.tile([128, 2, T], bf16, tag="avfm")

        xT_r = xT.rearrange("(kc p) t -> p kc t", p=128)
        nc.sync.dma_start(wk_sb, wk.rearrange("(kc p) f -> p kc f", p=128))
        for kc in range(KC):
            nc.sync.dma_start(xT_sb[:, kc], xT_r[:, kc])
        nc.sync.dma_start(wq_sb, wq.rearrange("(kc p) f -> p kc f", p=128))
        nc.sync.dma_start(wv_sb, wv.rearrange("(kc p) f -> p kc f", p=128))
        nc.sync.dma_start(cos_sb, cosT[:, :])
        nc.sync.dma_start(sin_sb, sinT[:, :])
        nc.sync.dma_start(mask_sb, masks[:, :])
        nc.sync.dma_start(id_sb, ident[:, :])
        nc.sync.dma_start(wo_sb, wo.rearrange("(fc p) d -> p fc d", p=128))
        nc.vector.memset(ones_col, 1.0)
        nc.vector.memset(v_sb[:, :, :, 64:65], 1.0)

        # ---- PSUM pools for the main body ----
        attn_ps = ExitStack()
        # "big" 2-bank slots serve the wide QK-proj groups, wide score tiles
        # and ss_tok; "sm" 1-bank slots serve narrow score tiles, filler
        # V-projections and filler QK-proj slices. Every use is transient.
        # avt: two single-buffered 65-wide accumulators per (h, qh); column 64
        # accumulates the softmax denominator (V's 65th column is ones).
        psBig = attn_ps.enter_context(tc.tile_pool(name="psBig", bufs=2, space="PSUM"))
        psSm = attn_ps.enter_context(tc.tile_pool(name="psSm", bufs=2, space="PSUM"))
        psAvt = attn_ps.enter_context(tc.tile_pool(name="psAvt", bufs=1, space="PSUM"))

        # ---- RMSNorm r (eps dropped: mean-square is O(1), below bf16 noise) ----
        # DVE: x^2 in kc-pair chunks (pipelines behind the xT chunk DMAs),
        # then an in-place kc tree-sum; the partition reduce + sqrt + recip +
        # cos/sin folds run split by column halves so the first half of the
        # rope tables is ready ~4us earlier.
        xsq_ctx = ExitStack()
        xsqp = xsq_ctx.enter_context(tc.tile_pool(name="xsqp", bufs=1))
        xsq = xsqp.tile([128, KC, T], bf16, tag="xsq")
        ss_bc = xsqp.tile([128, T], f32, tag="ssbc")
        sq_bc = xsqp.tile([128, T], bf16, tag="sqbc")
        r_bc = xsqp.tile([128, T], f32, tag="rbc")
        r_bcb = xsqp.tile([128, T], bf16, tag="rbcb")
        # running accumulator: each add consumes the just-arrived chunk pair,
        # so only [square, add, fold] trail the LAST xT chunk's DMA (the
        # critical path), instead of a full reduction tree
        for i in range(4):
            stt_mul(
                xsq[:, 2 * i : 2 * i + 2],
                xT_sb[:, 2 * i : 2 * i + 2],
                xT_sb[:, 2 * i : 2 * i + 2],
            )
            if i > 0:
                stt_add(
                    xsq[:, 0:2],
                    xsq[:, 0:2],
                    xsq[:, 2 * i : 2 * i + 2],
                )
        stt_add(xsq[:, 0], xsq[:, 0], xsq[:, 1])
        ssum = xsq[:, 0]

        def r_quarter(qt_):
            cs = slice(qt_ * 512, (qt_ + 1) * 512)
            nc.gpsimd.partition_all_reduce(
                ss_bc[:, cs], ssum[:, cs], 128, bass_isa.ReduceOp.add
            )
            nc.scalar.activation(sq_bc[:, cs], ss_bc[:, cs], Sqrt, scale=1.0 / DIM)
            nc.vector.reciprocal(r_bc[:, cs], sq_bc[:, cs])
            # bf16 copy of r (on the idle-early Act engine) lets the cos/sin
            # folds hit the DVE 4x mode
            nc.scalar.copy(out=r_bcb[:, cs], in_=r_bc[:, cs])
            stt_mul(cosr_sb[:, cs], cos_sb[:, cs], r_bcb[:, cs])
            stt_mul(sinr_sb[:, cs], sin_sb[:, cs], r_bcb[:, cs])

        def r_half(hf):
            r_quarter(2 * hf)
            r_quarter(2 * hf + 1)

        # ---- projection + rope helpers ----
        w_of = {0: (wq_sb, 0), 1: (wq_sb, 1), 2: (wk_sb, 0), 3: (wk_sb, 1)}

        def proj_cols(fidx, lo, width, copy_eng):
            """Project columns [lo, lo+width) of q/k block fidx into qk_sb."""
            wsb, fc = w_of[fidx]
            pool, tag = (psBig, "big") if width > 512 else (psSm, "sm")
            ps = pool.tile([128, width], f32, tag=tag, name=f"qkp_{fidx}_{lo}")
            for kc in range(KC):
                for s in range(0, width, 512):
                    w = min(512, width - s)
                    nc.tensor.matmul(
                        ps[:, s : s + w],
                        lhsT=wsb[:, kc, fc * 128 : (fc + 1) * 128],
                        rhs=xT_sb[:, kc, lo + s : lo + s + w],
                        start=(kc == 0),
                        stop=(kc == KC - 1),
                    )
            if copy_eng == "act":
                nc.scalar.copy(out=qk_sb[:, fidx, lo : lo + width], in_=ps)
            else:
                nc.vector.tensor_copy(out=qk_sb[:, fidx, lo : lo + width], in_=ps)

        def rope_cols(fidx, lo, width):
            """qk2[fidx] = qk[fidx]*cosr + rotate_half(qk[fidx])*sinr."""
            cs = slice(lo, lo + width)
            pt = work.tile([128, width], bf16, tag="perm", name=f"pm_{fidx}_{lo}")
            src = qk_sb[:, fidx, cs]
            for d0, s0 in ((0, 32), (32, 0), (64, 96), (96, 64)):
                nc.sync.dma_start(pt[d0 : d0 + 32], src[s0 : s0 + 32])
            t2 = work.tile([128, width], bf16, tag="t2", name=f"t2_{fidx}_{lo}")
            stt_mul(t2, qk_sb[:, fidx, cs], cosr_sb[:, cs])
            t1 = work.tile([128, width], bf16, tag="t1", name=f"t1_{fidx}_{lo}")
            stt_mul(t1, pt, sinr_sb[:, cs])
            stt_add(qk2_sb[:, fidx, cs], t2, t1)

        def v_group(tt, hp):
            """Project V for token block tt, head pair hp, and apply r."""
            psv = psSm.tile([128, 128], f32, tag="sm", name=f"psv_{tt}_{hp}")
            for kc in range(KC):
                nc.tensor.matmul(
                    psv,
                    lhsT=xT_sb[:, kc, tt * 128 : (tt + 1) * 128],
                    rhs=wv_sb[:, kc, hp * 128 : (hp + 1) * 128],
                    start=(kc == 0),
                    stop=(kc == KC - 1),
                )
            nc.vector.tensor_scalar_mul(
                v_sb[:, tt, 2 * hp : 2 * hp + 2, 0:64],
                psv.rearrange("p (h d) -> p h d", h=2),
                r_tok[:, tt : tt + 1],
            )

        # Q/K projections for heads 0,1 (fidx 2=k, 0=q). PE starts on these as
        # xT chunks arrive; Act does the PSUM->SBUF copies (it is idle until
        # the first exp); DVE owns the RMS chain so it never blocks a copy.
        for fidx in (2, 0):
            proj_cols(fidx, 0, 1024, "act")
            proj_cols(fidx, 1024, 1024, "act")
        # token-major r (for V): per-token sums via 16 one-row ones-matmuls.
        # Emitted after the projections so they don't block the PE queue head
        # (they wait on the full xsq tree).
        ss_tok = psBig.tile([128, 16], f32, tag="big", name="ss_tok")
        for tt in range(16):
            nc.tensor.matmul(
                ss_tok[:, tt : tt + 1],
                lhsT=ssum[:, tt * 128 : (tt + 1) * 128],
                rhs=ones_col,
                start=True,
                stop=True,
            )
        nc.scalar.activation(sq_tok, ss_tok, Sqrt, scale=1.0 / DIM)
        nc.vector.reciprocal(r_tok, sq_tok)
        # r + rope, half 0 first so h0-qh0 attention can start early
        r_half(0)
        rope_cols(2, 0, 1024)
        rope_cols(0, 0, 1024)
        r_half(1)
        if debug:
            nc.sync.dma_start(d_rbc[:, :], r_bc)
        xsq_ctx.close()
        expp = ctx.enter_context(tc.tile_pool(name="expp", bufs=17))
        rope_cols(2, 1024, 1024)
        rope_cols(0, 1024, 1024)
        # bridge the PE gap between the projections and the first score matmul
        v_group(0, 0)
        v_group(1, 0)
        v_group(2, 0)
        v_group(3, 0)

        # fillers: one popped per attention kb iteration, sized ~0.4us each so
        # they never delay the next score matmul by more than one exp. V-hp0
        # (heads 0,1) front-runs the h0/h1 kb sweeps; the fidx 3/1 projections
        # and ropes complete during h1; V-hp1 front-runs h2 (lag-4 covers the
        # small pop-vs-use slack).
        fillers = [(lambda tt=tt: v_group(tt, 0)) for tt in range(4, 16)]
        for fidx in (3, 1):
            for e in range(8):
                fillers.append(
                    lambda f=fidx, e=e: proj_cols(f, e * 128, 128, "dve")
                )
            fillers.append(lambda f=fidx: rope_cols(f, 0, 1024))
            for e in range(8, 16):
                fillers.append(
                    lambda f=fidx, e=e: proj_cols(f, e * 128, 128, "dve")
                )
            if fidx == 1:
                # slip the first two V-hp1 groups ahead of the final rope
                # (which has 8 iterations of deadline slack into h2) so the
                # h2 AV matmuls get 2 iterations of V-scale slack
                fillers.append(lambda: v_group(0, 1))
                fillers.append(lambda: v_group(1, 1))
            fillers.append(lambda f=fidx: rope_cols(f, 1024, 1024))
        fillers += [(lambda tt=tt: v_group(tt, 1)) for tt in range(2, 16)]

        # ---- attention ----
        # pend holds (entry, fin), one entry per query tile jt. Each entry
        # emits the FULL kb-accumulation for that q-tile as consecutive
        # matmuls — PSUM accumulation groups must be contiguous per bank on
        # real HW. Entries lag the exp stream by one key block so they don't
        # wait on an exp semaphore, and sweep finalization (normalize +
        # PE transpose to feature-major) rides the same queue past sweep
        # boundaries so head transitions never stall the PE.
        pend = []

        def avt_mms(h, qh, jt, exs, avts):
            qlo = qh * 1024
            jl = jt - qh * 8
            for kb in range(jt + 1):
                ex, c0 = exs[kb]
                off = jt * 128 - qlo - c0
                nc.tensor.matmul(
                    avts[jl // 4][:, jl % 4, :],
                    lhsT=ex[:, off : off + 128],
                    rhs=v_sb[:, kb, h],
                    start=(kb == 0),
                    stop=(kb == jt),
                )

        def pend_pop():
            entry, fin = pend.pop(0)
            avt_mms(*entry)
            if fin is not None:
                fin()

        def make_finalize(h, qh, avts):
            def fin():
                for half in range(2):
                    rec = vecs.tile(
                        [128, 4],
                        f32,
                        tag=f"rec{half}",
                        name=f"rec_{h}_{qh}_{half}",
                    )
                    nc.vector.reciprocal(rec, avts[half][:, :, 64])
                    nc.vector.tensor_tensor(
                        av_tok[:, h, qh * 8 + 4 * half : qh * 8 + 4 * half + 4, :],
                        avts[half][:, :, 0:64],
                        rec.broadcast_to([128, 4, DIM_HEAD]),
                        mult,
                    )
                # token-major -> feature-major via PE identity transposes
                r0 = (h % 2) * 64
                for half in range(2):
                    ptr = psSm.tile(
                        [64, 4, 128], bf16, tag="sm", name=f"tr_{h}_{qh}_{half}"
                    )
                    for j in range(4):
                        nc.tensor.transpose(
                            ptr[:, j, :],
                            av_tok[:, h, qh * 8 + 4 * half + j, :],
                            id_sb,
                        )
                    nc.vector.tensor_copy(
                        out=av_fm[
                            r0 : r0 + 64,
                            h // 2,
                            qh * 1024 + half * 512 : qh * 1024 + half * 512 + 512,
                        ],
                        in_=ptr,
                    )

            return fin

        def attention(h, qh):
            qt = qk2_sb[:, 0 if h < 2 else 1]
            kt = qk2_sb[:, 2 if h < 2 else 3]
            rows = slice((h % 2) * 64, (h % 2) * 64 + 64)
            qlo = qh * 1024
            nkb = 8 * (qh + 1)

            def sc_exp(kb, mid=None):
                """Emit the score matmuls + exp (+ mask) for key block kb.
                `mid` (the pend pops) runs between the matmuls and the exp
                tile allocation — popped entries read old exp tiles whose
                pool slots the new tile reuses."""
                c0 = max(kb * 128 - qlo, 0)
                W = 1024 - c0
                if W > 512:
                    sc = psBig.tile(
                        [128, 1024], f32, tag="big", name=f"sc_{h}_{qh}_{kb}"
                    )
                else:
                    sc = psSm.tile(
                        [128, 512], f32, tag="sm", name=f"sc_{h}_{qh}_{kb}"
                    )
                for o in range(0, W, 512):
                    w = min(512, W - o)
                    nc.tensor.matmul(
                        sc[:, o : o + w],
                        lhsT=kt[rows, kb * 128 : (kb + 1) * 128],
                        rhs=qt[rows, qlo + c0 + o : qlo + c0 + o + w],
                        start=True,
                        stop=True,
                    )
                if mid is not None:
                    mid()
                ex = expp.tile(
                    [128, 1024], bf16, tag="exp", name=f"ex_{h}_{qh}_{kb}"
                )
                nc.scalar.activation(ex[:, 0:W], sc[:, 0:W], Exp)
                if kb * 128 >= qlo:
                    stt_mul(ex[:, 0:128], ex[:, 0:128], mask_sb)
                return ex, c0

            def pops():
                while len(pend) > 1:
                    pend_pop()

            # drain the previous sweep (its last q-tile group + finalize)
            # before its exp tiles' pool slots get reused below
            while pend:
                pend_pop()
            avts = (
                psAvt.tile([128, 4, 65], f32, tag="avt_a", name=f"avta_{h}_{qh}"),
                psAvt.tile([128, 4, 65], f32, tag="avt_b", name=f"avtb_{h}_{qh}"),
            )
            exs = [sc_exp(0)]
            if fillers:
                fillers.pop(0)()
            if qh == 0:
                pend.append(((h, qh, 0, exs, avts), None))
            for kb in range(1, nkb):
                exs.append(sc_exp(kb, mid=pops))
                if fillers:
                    fillers.pop(0)()
                if kb >= qh * 8:
                    jt = kb
                    fin = make_finalize(h, qh, avts) if kb == nkb - 1 else None
                    pend.append(((h, qh, jt, exs, avts), fin))

        # ---- out projection chunks (partial over heads; host sums) ----
        out_r = out.rearrange("(do p) t -> p do t", p=128)
        ob_tiles = {}

        def outproj_chunk(do, ch, s2, spread=False):
            """512 output columns for output-row block do, column half ch.
            spread=True (tail only, when attention no longer needs PSUM)
            alternates chunks across the sm and big pools so the rotation is
            two output-blocks deep instead of lockstepping on one slot."""
            if spread and (2 * do + s2) % 2 == 1:
                po = psBig.tile(
                    [128, 512], f32, tag="big", name=f"po_{do}_{ch}_{s2}"
                )
            else:
                po = psSm.tile(
                    [128, 512], f32, tag="sm", name=f"po_{do}_{ch}_{s2}"
                )
            cs = slice(ch * 1024 + s2 * 512, ch * 1024 + s2 * 512 + 512)
            for hp in range(2):
                nc.tensor.matmul(
                    po,
                    lhsT=wo_sb[:, hp, do * 128 : (do + 1) * 128],
                    rhs=av_fm[:, hp, cs],
                    start=(hp == 0),
                    stop=(hp == 1),
                )
            if s2 == 0:
                ob_tiles[(do, ch)] = work.tile(
                    [128, 1024], bf16, tag="ob", name=f"ob_{do}_{ch}"
                )
            ob = ob_tiles[(do, ch)]
            if (do + s2) % 2 == 0:
                nc.scalar.copy(out=ob[:, s2 * 512 : (s2 + 1) * 512], in_=po)
            else:
                nc.vector.tensor_copy(
                    out=ob[:, s2 * 512 : (s2 + 1) * 512], in_=po
                )
            if s2 == 1:
                nc.sync.dma_start(
                    out_r[:, do, ch * 1024 : (ch + 1) * 1024], ob
                )

        # h1-qh0 runs second: it needs only the half-0 rope tables (ready
        # early), filling the Act gap while the half-1 rope chain still
        # drains on the DVE; h0-qh1 follows once those tables exist. Filler
        # deadlines are positional and the pre-h2 iteration count is
        # unchanged, so the pop schedule is identical.
        for h, qh in ((0, 0), (1, 0), (0, 1), (1, 1), (2, 0), (3, 0)):
            attention(h, qh)
        # out-projection for query half 0 rides the (2,1)+(3,1) filler slots
        # (all heads' qh0 av_fm is written once (3,0) finalizes at the (2,1)
        # sweep entry) - two Act-heavy sweeps give it twice the slack
        for do in range(8):
            for s2 in range(2):
                fillers.append(lambda d=do, s=s2: outproj_chunk(d, 0, s))
        attention(2, 1)
        attention(3, 1)
        while fillers:
            fillers.pop(0)()
        while pend:
            pend_pop()
        for do in range(8):
            for s2 in range(2):
                outproj_chunk(do, 1, s2, spread=True)
        attn_ps.close()
        if debug:
            nc.sync.dma_start(d_rbc[:, :], r_bc)
            nc.sync.dma_start(d_qk.rearrange("p (f t) -> p f t", f=4), qk_sb)
            nc.sync.dma_start(d_qk2.rearrange("p (f t) -> p f t", f=4), qk2_sb)
            nc.sync.dma_start(
                d_v.rearrange("p (a b c) -> p a b c", a=16, b=HPC), v_sb
            )
            nc.sync.dma_start(
                d_avtok.rearrange("p (a b c) -> p a b c", a=HPC, b=16), av_tok
            )
            nc.sync.dma_start(d_avfm.rearrange("p (a t) -> p a t", a=2), av_fm)
    nc.compile()
    return nc


def _host_inputs(x, norm_w, w_qkv, w_o, sin, cos):
    """Build the 8 per-core input maps (all bf16)."""
    n = T
    w_eff = np.asarray(w_qkv, np.float64) * np.asarray(norm_w, np.float64)[:, None]
    sin_n = np.asarray(sin, np.float32)[:n]  # [T, 64]
    cos_n = np.asarray(cos, np.float32)[:n]
    sign = np.concatenate([-np.ones(32, np.float32), np.ones(32, np.float32)])
    cos_tile = np.tile(cos_n.T, (2, 1))  # [128, T]
    sin_tile = np.tile((sin_n * sign[None, :]).T, (2, 1))  # [128, T]
    ql = np.arange(128)[None, :]
    key = np.arange(128)[:, None]
    masks = (ql >= key).astype(np.float32)
    ident_np = np.eye(128, dtype=np.float32)

    in_maps = []
    for c in range(8):
        b, g = c // 4, c % 4
        fs = slice(g * F, (g + 1) * F)
        in_maps.append(
            {
                "xT": np.ascontiguousarray(np.asarray(x, np.float32)[b].T).astype(BF16),
                "wq": (w_eff[:, 0:DIM][:, fs] * (DIM_HEAD ** -0.5)).astype(BF16),
                "wk": w_eff[:, DIM : 2 * DIM][:, fs].astype(BF16),
                "wv": w_eff[:, 2 * DIM : 3 * DIM][:, fs].astype(BF16),
                "wo": np.asarray(w_o, np.float32)[fs, :].astype(BF16),
                "cosT": cos_tile.astype(BF16),
                "sinT": sin_tile.astype(BF16),
                "masks": masks.astype(BF16),
                "ident": ident_np.astype(BF16),
            }
        )
    return in_maps


def kernel(x, norm_w, w_qkv, w_o, b_o, sin, cos):
    from concourse.bass_utils import run_bass_kernel_spmd

    if "nc" not in _NC_CACHE:
        _NC_CACHE["nc"] = _build_nc()
    nc = _NC_CACHE["nc"]
    in_maps = _host_inputs(x, norm_w, w_qkv, w_o, sin, cos)
    trace = bool(int(os.environ.get("KERNEL_TRACE", "0")))
    res = run_bass_kernel_spmd(nc, in_maps, core_ids=list(range(8)), trace=trace)
    if trace and res.exec_time_ns is not None:
        print(f"HW exec time: {res.exec_time_ns} ns")
    outs = [r["out"].astype(np.float32) for r in res.results]  # [1024, T] fm
    b_o = np.asarray(b_o, np.float32)
    full = np.empty((B, T, DIM), np.float32)
    for b in range(B):
        acc = outs[b * 4] + outs[b * 4 + 1] + outs[b * 4 + 2] + outs[b * 4 + 3]
        full[b] = acc.T + b_o[None, :]
    return full


# revision 77
# speedup vs baseline: 1.0135x; 1.0135x over previous
"""Trainium2 8-core kernel for RMSNorm -> QKV -> RoPE -> causal SDPA -> out-proj.

Sharding: core c = b*4 + g handles batch b (of 2) and heads 4g..4g+3 (of 16).
Each core computes a partial out-projection [dim, tokens]; the host sums the
4 head-group partials per batch and adds b_o.

Key layout/scheduling choices (cost model: matmul time = out-free-size rows,
vector/scalar op time = max free size):
- RMSNorm r is folded into BOTH Q and K rope tables (cosr/sinr = cos/sin * r),
  so softmax exp needs no per-key scale; V gets r via a token-major
  tensor_scalar. r itself comes from a DVE square + kc-tree-sum + gpsimd
  partition_all_reduce (no PE time), plus 16 one-row ones-matmuls for the
  token-major r_tok.
- rotate-half is 4 partition-permuting SBUF->SBUF DMAs per projection block
  (no PE perm matmul, no PSUM).
- Scores are computed key-major [128 keys, q] with exact causal starts.
- AV is computed TRANSPOSED: lhsT = exp tile [128 keys, 128 q], rhs = V
  [128 keys, 65] -> avt [q, d] at 65 rows/matmul (the PE streams only the
  65-wide rhs). Column 64 accumulates the softmax denominator (V's 65th
  column is ones); normalization is a per-partition reciprocal + one
  stride-0-broadcast multiply. Each q-tile's key-block accumulation is
  emitted as CONSECUTIVE matmuls: on real HW a PSUM bank supports only one
  open accumulation group at a time (interleaved-region accumulation that
  the interpreter accepts silently corrupts on silicon).
- av (token-major) -> feature-major for the out-projection via PE identity
  transposes (the XBAR dma_start_transpose's blocked-3D mode does not match
  interpreter semantics on HW).
- V projection and the second half of the QKV projections are emitted as
  "filler" work inside the attention kb loops so the PE stays busy while the
  Activation engine (the attention-era bottleneck) streams exp; AV groups
  lag the exp stream by one key block and sweep finalization rides the same
  deferred queue past head boundaries.
"""

import os

import numpy as np
import ml_dtypes

BF16 = ml_dtypes.bfloat16

DIM = 1024
HEADS = 16
DIM_HEAD = 64
T = 2048  # tokens per batch
B = 2
HPC = 4  # heads per core
F = HPC * DIM_HEAD  # 256 per-core head width
KC = DIM // 128  # 8 contraction chunks

_NC_CACHE = {}


def _build_nc():
    import concourse.bacc as bacc
    import concourse.mybir as mybir
    import concourse.tile as tile
    from concourse import bass_isa
    from contextlib import ExitStack

    f32 = mybir.dt.float32
    bf16 = mybir.dt.bfloat16
    nc = bacc.Bacc()

    xT = nc.declare_dram_parameter("xT", [DIM, T], bf16, isOutput=False)
    wq = nc.declare_dram_parameter("wq", [DIM, F], bf16, isOutput=False)
    wk = nc.declare_dram_parameter("wk", [DIM, F], bf16, isOutput=False)
    wv = nc.declare_dram_parameter("wv", [DIM, F], bf16, isOutput=False)
    wo = nc.declare_dram_parameter("wo", [F, DIM], bf16, isOutput=False)
    cosT = nc.declare_dram_parameter("cosT", [128, T], bf16, isOutput=False)
    sinT = nc.declare_dram_parameter("sinT", [128, T], bf16, isOutput=False)
    masks = nc.declare_dram_parameter("masks", [128, 128], bf16, isOutput=False)
    ident = nc.declare_dram_parameter("ident", [128, 128], bf16, isOutput=False)
    out = nc.declare_dram_parameter("out", [DIM, T], bf16, isOutput=True)
    debug = bool(int(os.environ.get("KERNEL_DEBUG", "0")))
    if debug:
        d_rbc = nc.declare_dram_parameter("d_rbc", [128, T], mybir_dt_f32 := __import__("concourse.mybir", fromlist=["dt"]).dt.float32, isOutput=True)
        d_qk = nc.declare_dram_parameter("d_qk", [128, 4 * T], bf16, isOutput=True)
        d_qk2 = nc.declare_dram_parameter("d_qk2", [128, 4 * T], bf16, isOutput=True)
        d_v = nc.declare_dram_parameter("d_v", [128, 16 * HPC * 65], bf16, isOutput=True)
        d_avtok = nc.declare_dram_parameter("d_avtok", [128, HPC * 16 * 64], bf16, isOutput=True)
        d_avfm = nc.declare_dram_parameter("d_avfm", [128, 2 * T], bf16, isOutput=True)

    Exp = mybir.ActivationFunctionType.Exp
    Sqrt = mybir.ActivationFunctionType.Sqrt
    mult = mybir.AluOpType.mult
    add = mybir.AluOpType.add

    nc_ = nc

    def stt_mul(out, a, b):
        return nc_.vector.tensor_mul(out, a, b)

    def stt_add(out, a, b):
        return nc_.vector.tensor_tensor(out, a, b, add)

    with ExitStack() as ctx:
        tc = ctx.enter_context(tile.TileContext(nc))
        consts = ctx.enter_context(tc.tile_pool(name="consts", bufs=1))
        persist = ctx.enter_context(tc.tile_pool(name="persist", bufs=1))
        work = ctx.enter_context(tc.tile_pool(name="work", bufs=2))
        vecs = ctx.enter_context(tc.tile_pool(name="vecs", bufs=2))

        # ---- constants / inputs ----
        wq_sb = consts.tile([128, KC, F], bf16, tag="wq")
        wk_sb = consts.tile([128, KC, F], bf16, tag="wk")
        wv_sb = consts.tile([128, KC, F], bf16, tag="wv")
        wo_sb = consts.tile([128, 2, DIM], bf16, tag="wo")
        cos_sb = consts.tile([128, T], bf16, tag="cos")
        sin_sb = consts.tile([128, T], bf16, tag="sin")
        mask_sb = consts.tile([128, 128], bf16, tag="mask")
        ones_col = consts.tile([128, 1], bf16, tag="onesc")
        id_sb = consts.tile([128, 128], bf16, tag="ident")

        xT_sb = persist.tile([128, KC, T], bf16, tag="xT")
        qk_sb = persist.tile([128, 4, T], bf16, tag="qk")  # raw projections
        qk2_sb = persist.tile([128, 4, T], bf16, tag="qk2")  # roped projections
        v_sb = persist.tile([128, 16, HPC, 65], bf16, tag="v")
        cosr_sb = persist.tile([128, T], bf16, tag="cosr")
        sinr_sb = persist.tile([128, T], bf16, tag="sinr")
        sq_tok = persist.tile([128, 16], f32, tag="sqtok")
        r_tok = persist.tile([128, 16], f32, tag="rtok")
        av_tok = persist.tile([128, HPC, 16, DIM_HEAD], bf16, tag="avtok")
        av_fm = persist.tile([128, 2, T], bf16, tag="avfm")

        xT_r = xT.rearrange("(kc p) t -> p kc t", p=128)
        nc.sync.dma_start(wk_sb, wk.rearrange("(kc p) f -> p kc f", p=128))
        for kc in range(KC):
            nc.sync.dma_start(xT_sb[:, kc], xT_r[:, kc])
        nc.sync.dma_start(wq_sb, wq.rearrange("(kc p) f -> p kc f", p=128))
        nc.sync.dma_start(wv_sb, wv.rearrange("(kc p) f -> p kc f", p=128))
        nc.sync.dma_start(cos_sb, cosT[:, :])
        nc.sync.dma_start(sin_sb, sinT[:, :])
        nc.sync.dma_start(mask_sb, masks[:, :])
        nc.sync.dma_start(id_sb, ident[:, :])
        nc.sync.dma_start(wo_sb, wo.rearrange("(fc p) d -> p fc d", p=128))
        nc.vector.memset(ones_col, 1.0)
        nc.vector.memset(v_sb[:, :, :, 64:65], 1.0)

        # ---- PSUM pools for the main body ----
        attn_ps = ExitStack()
        # "big" 2-bank slots serve the wide QK-proj groups, wide score tiles
        # and ss_tok; "sm" 1-bank slots serve narrow score tiles, filler
        # V-projections and filler QK-proj slices. Every use is transient.
        # avt: two single-buffered 65-wide accumulators per (h, qh); column 64
        # accumulates the softmax denominator (V's 65th column is ones).
        psBig = attn_ps.enter_context(tc.tile_pool(name="psBig", bufs=2, space="PSUM"))
        psSm = attn_ps.enter_context(tc.tile_pool(name="psSm", bufs=2, space="PSUM"))
        psAvt = attn_ps.enter_context(tc.tile_pool(name="psAvt", bufs=1, space="PSUM"))

        # ---- RMSNorm r (eps dropped: mean-square is O(1), below bf16 noise) ----
        # DVE: x^2 in kc-pair chunks (pipelines behind the xT chunk DMAs),
        # then an in-place kc tree-sum; the partition reduce + sqrt + recip +
        # cos/sin folds run split by column halves so the first half of the
        # rope tables is ready ~4us earlier.
        xsq_ctx = ExitStack()
        xsqp = xsq_ctx.enter_context(tc.tile_pool(name="xsqp", bufs=1))
        xsq = xsqp.tile([128, KC, T], bf16, tag="xsq")
        ss_bc = xsqp.tile([128, T], f32, tag="ssbc")
        sq_bc = xsqp.tile([128, T], bf16, tag="sqbc")
        r_bc = xsqp.tile([128, T], f32, tag="rbc")
        r_bcb = xsqp.tile([128, T], bf16, tag="rbcb")
        # running accumulator: each add consumes the just-arrived chunk pair,
        # so only [square, add, fold] trail the LAST xT chunk's DMA (the
        # critical path), instead of a full reduction tree
        for i in range(4):
            stt_mul(
                xsq[:, 2 * i : 2 * i + 2],
                xT_sb[:, 2 * i : 2 * i + 2],
                xT_sb[:, 2 * i : 2 * i + 2],
            )
            if i > 0:
                stt_add(
                    xsq[:, 0:2],
                    xsq[:, 0:2],
                    xsq[:, 2 * i : 2 * i + 2],
                )
        stt_add(xsq[:, 0], xsq[:, 0], xsq[:, 1])
        ssum = xsq[:, 0]

        def r_quarter(qt_):
            cs = slice(qt_ * 512, (qt_ + 1) * 512)
            nc.gpsimd.partition_all_reduce(
                ss_bc[:, cs], ssum[:, cs], 128, bass_isa.ReduceOp.add
            )
            nc.scalar.activation(sq_bc[:, cs], ss_bc[:, cs], Sqrt, scale=1.0 / DIM)
            nc.vector.reciprocal(r_bc[:, cs], sq_bc[:, cs])
            # bf16 copy of r (on the idle-early Act engine) lets the cos/sin
            # folds hit the DVE 4x mode
            nc.scalar.copy(out=r_bcb[:, cs], in_=r_bc[:, cs])
            stt_mul(cosr_sb[:, cs], cos_sb[:, cs], r_bcb[:, cs])
            stt_mul(sinr_sb[:, cs], sin_sb[:, cs], r_bcb[:, cs])

        def r_half(hf):
            r_quarter(2 * hf)
            r_quarter(2 * hf + 1)

        # ---- projection + rope helpers ----
        w_of = {0: (wq_sb, 0), 1: (wq_sb, 1), 2: (wk_sb, 0), 3: (wk_sb, 1)}

        def proj_cols(fidx, lo, width, copy_eng):
            """Project columns [lo, lo+width) of q/k block fidx into qk_sb."""
            wsb, fc = w_of[fidx]
            pool, tag = (psBig, "big") if width > 512 else (psSm, "sm")
            ps = pool.tile([128, width], f32, tag=tag, name=f"qkp_{fidx}_{lo}")
            for kc in range(KC):
                for s in range(0, width, 512):
                    w = min(512, width - s)
                    nc.tensor.matmul(
                        ps[:, s : s + w],
                        lhsT=wsb[:, kc, fc * 128 : (fc + 1) * 128],
                        rhs=xT_sb[:, kc, lo + s : lo + s + w],
                        start=(kc == 0),
                        stop=(kc == KC - 1),
                    )
            if copy_eng == "act":
                nc.scalar.copy(out=qk_sb[:, fidx, lo : lo + width], in_=ps)
            else:
                nc.vector.tensor_copy(out=qk_sb[:, fidx, lo : lo + width], in_=ps)

        def rope_cols(fidx, lo, width):
            """qk2[fidx] = qk[fidx]*cosr + rotate_half(qk[fidx])*sinr."""
            cs = slice(lo, lo + width)
            pt = work.tile([128, width], bf16, tag="perm", name=f"pm_{fidx}_{lo}")
            src = qk_sb[:, fidx, cs]
            for d0, s0 in ((0, 32), (32, 0), (64, 96), (96, 64)):
                nc.sync.dma_start(pt[d0 : d0 + 32], src[s0 : s0 + 32])
            t2 = work.tile([128, width], bf16, tag="t2", name=f"t2_{fidx}_{lo}")
            stt_mul(t2, qk_sb[:, fidx, cs], cosr_sb[:, cs])
            t1 = work.tile([128, width], bf16, tag="t1", name=f"t1_{fidx}_{lo}")
            stt_mul(t1, pt, sinr_sb[:, cs])
            stt_add(qk2_sb[:, fidx, cs], t2, t1)

        def v_group(tt, hp):
            """Project V for token block tt, head pair hp, and apply r."""
            psv = psSm.tile([128, 128], f32, tag="sm", name=f"psv_{tt}_{hp}")
            for kc in range(KC):
                nc.tensor.matmul(
                    psv,
                    lhsT=xT_sb[:, kc, tt * 128 : (tt + 1) * 128],
                    rhs=wv_sb[:, kc, hp * 128 : (hp + 1) * 128],
                    start=(kc == 0),
                    stop=(kc == KC - 1),
                )
            nc.vector.tensor_scalar_mul(
                v_sb[:, tt, 2 * hp : 2 * hp + 2, 0:64],
                psv.rearrange("p (h d) -> p h d", h=2),
                r_tok[:, tt : tt + 1],
            )

        # Q/K projections for heads 0,1 (fidx 2=k, 0=q). PE starts on these as
        # xT chunks arrive; Act does the PSUM->SBUF copies (it is idle until
        # the first exp); DVE owns the RMS chain so it never blocks a copy.
        for fidx in (2, 0):
            proj_cols(fidx, 0, 1024, "act")
            proj_cols(fidx, 1024, 1024, "act")
        # token-major r (for V): per-token sums via 16 one-row ones-matmuls.
        # Emitted after the projections so they don't block the PE queue head
        # (they wait on the full xsq tree).
        ss_tok = psBig.tile([128, 16], f32, tag="big", name="ss_tok")
        for tt in range(16):
            nc.tensor.matmul(
                ss_tok[:, tt : tt + 1],
                lhsT=ssum[:, tt * 128 : (tt + 1) * 128],
                rhs=ones_col,
                start=True,
                stop=True,
            )
        nc.scalar.activation(sq_tok, ss_tok, Sqrt, scale=1.0 / DIM)
        nc.vector.reciprocal(r_tok, sq_tok)
        # r + rope, half 0 first so h0-qh0 attention can start early
        r_half(0)
        rope_cols(2, 0, 1024)
        rope_cols(0, 0, 1024)
        r_half(1)
        if debug:
            nc.sync.dma_start(d_rbc[:, :], r_bc)
        xsq_ctx.close()
        expp = ctx.enter_context(tc.tile_pool(name="expp", bufs=17))
        rope_cols(2, 1024, 1024)
        rope_cols(0, 1024, 1024)
        # bridge the PE gap between the projections and the first score matmul
        v_group(0, 0)
        v_group(1, 0)
        v_group(2, 0)
        v_group(3, 0)

        # fillers: one popped per attention kb iteration, sized ~0.4us each so
        # they never delay the next score matmul by more than one exp. V-hp0
        # (heads 0,1) front-runs the h0/h1 kb sweeps; the fidx 3/1 projections
        # and ropes complete during h1; V-hp1 front-runs h2 (lag-4 covers the
        # small pop-vs-use slack).
        fillers = [(lambda tt=tt: v_group(tt, 0)) for tt in range(4, 16)]
        for fidx in (3, 1):
            for e in range(8):
                fillers.append(
                    lambda f=fidx, e=e: proj_cols(f, e * 128, 128, "dve")
                )
            fillers.append(lambda f=fidx: rope_cols(f, 0, 1024))
            for e in range(8, 16):
                fillers.append(
                    lambda f=fidx, e=e: proj_cols(f, e * 128, 128, "dve")
                )
            if fidx == 1:
                # slip the first two V-hp1 groups ahead of the final rope
                # (which has 8 iterations of deadline slack into h2) so the
                # h2 AV matmuls get 2 iterations of V-scale slack
                fillers.append(lambda: v_group(0, 1))
                fillers.append(lambda: v_group(1, 1))
            fillers.append(lambda f=fidx: rope_cols(f, 1024, 1024))
        fillers += [(lambda tt=tt: v_group(tt, 1)) for tt in range(2, 16)]

        # ---- attention ----
        # pend holds (entry, fin), one entry per query tile jt. Each entry
        # emits the FULL kb-accumulation for that q-tile as consecutive
        # matmuls — PSUM accumulation groups must be contiguous per bank on
        # real HW. Entries lag the exp stream by one key block so they don't
        # wait on an exp semaphore, and sweep finalization (normalize +
        # PE transpose to feature-major) rides the same queue past sweep
        # boundaries so head transitions never stall the PE.
        pend = []

        def avt_mms(h, qh, jt, exs, avts):
            qlo = qh * 1024
            jl = jt - qh * 8
            for kb in range(jt + 1):
                ex, c0 = exs[kb]
                off = jt * 128 - qlo - c0
                nc.tensor.matmul(
                    avts[jl // 4][:, jl % 4, :],
                    lhsT=ex[:, off : off + 128],
                    rhs=v_sb[:, kb, h],
                    start=(kb == 0),
                    stop=(kb == jt),
                )

        def pend_pop():
            entry, fin = pend.pop(0)
            avt_mms(*entry)
            if fin is not None:
                fin()

        def make_finalize(h, qh, avts):
            def fin():
                for half in range(2):
                    rec = vecs.tile(
                        [128, 4],
                        f32,
                        tag=f"rec{half}",
                        name=f"rec_{h}_{qh}_{half}",
                    )
                    nc.vector.reciprocal(rec, avts[half][:, :, 64])
                    nc.vector.tensor_tensor(
                        av_tok[:, h, qh * 8 + 4 * half : qh * 8 + 4 * half + 4, :],
                        avts[half][:, :, 0:64],
                        rec.broadcast_to([128, 4, DIM_HEAD]),
                        mult,
                    )
                # token-major -> feature-major via PE identity transposes
                r0 = (h % 2) * 64
                for half in range(2):
                    ptr = psSm.tile(
                        [64, 4, 128], bf16, tag="sm", name=f"tr_{h}_{qh}_{half}"
                    )
                    for j in range(4):
                        nc.tensor.transpose(
                            ptr[:, j, :],
                            av_tok[:, h, qh * 8 + 4 * half + j, :],
                            id_sb,
                        )
                    nc.vector.tensor_copy(
                        out=av_fm[
                            r0 : r0 + 64,
                            h // 2,
                            qh * 1024 + half * 512 : qh * 1024 + half * 512 + 512,
                        ],
                        in_=ptr,
                    )

            return fin

        def attention(h, qh):
            qt = qk2_sb[:, 0 if h < 2 else 1]
            kt = qk2_sb[:, 2 if h < 2 else 3]
            rows = slice((h % 2) * 64, (h % 2) * 64 + 64)
            qlo = qh * 1024
            nkb = 8 * (qh + 1)

            def sc_exp(kb, mid=None):
                """Emit the score matmuls + exp (+ mask) for key block kb.
                `mid` (the pend pops) runs between the matmuls and the exp
                tile allocation — popped entries read old exp tiles whose
                pool slots the new tile reuses."""
                c0 = max(kb * 128 - qlo, 0)
                W = 1024 - c0
                if W > 512:
                    sc = psBig.tile(
                        [128, 1024], f32, tag="big", name=f"sc_{h}_{qh}_{kb}"
                    )
                else:
                    sc = psSm.tile(
                        [128, 512], f32, tag="sm", name=f"sc_{h}_{qh}_{kb}"
                    )
                for o in range(0, W, 512):
                    w = min(512, W - o)
                    nc.tensor.matmul(
                        sc[:, o : o + w],
                        lhsT=kt[rows, kb * 128 : (kb + 1) * 128],
                        rhs=qt[rows, qlo + c0 + o : qlo + c0 + o + w],
                        start=True,
                        stop=True,
                    )
                if mid is not None:
                    mid()
                ex = expp.tile(
                    [128, 1024], bf16, tag="exp", name=f"ex_{h}_{qh}_{kb}"
                )
                nc.scalar.activation(ex[:, 0:W], sc[:, 0:W], Exp)
                if kb * 128 >= qlo:
                    stt_mul(ex[:, 0:128], ex[:, 0:128], mask_sb)
                return ex, c0

            def pops():
                while len(pend) > 1:
                    pend_pop()

            # drain the previous sweep (its last q-tile group + finalize)
            # before its exp tiles' pool slots get reused below
            while pend:
                pend_pop()
            avts = (
                psAvt.tile([128, 4, 65], f32, tag="avt_a", name=f"avta_{h}_{qh}"),
                psAvt.tile([128, 4, 65], f32, tag="avt_b", name=f"avtb_{h}_{qh}"),
            )
            exs = [sc_exp(0)]
            if fillers:
                fillers.pop(0)()
            if qh == 0:
                pend.append(((h, qh, 0, exs, avts), None))
            for kb in range(1, nkb):
                exs.append(sc_exp(kb, mid=pops))
                if fillers:
                    fillers.pop(0)()
                if kb >= qh * 8:
                    jt = kb
                    fin = make_finalize(h, qh, avts) if kb == nkb - 1 else None
                    pend.append(((h, qh, jt, exs, avts), fin))

        # ---- out projection chunks (partial over heads; host sums) ----
        out_r = out.rearrange("(do p) t -> p do t", p=128)
        ob_tiles = {}

        def outproj_chunk(do, ch, s2, spread=False):
            """512 output columns for output-row block do, column half ch.
            spread=True (tail only, when attention no longer needs PSUM)
            alternates chunks across the sm and big pools so the rotation is
            two output-blocks deep instead of lockstepping on one slot."""
            if spread and (2 * do + s2) % 2 == 1:
                po = psBig.tile(
                    [128, 512], f32, tag="big", name=f"po_{do}_{ch}_{s2}"
                )
            else:
                po = psSm.tile(
                    [128, 512], f32, tag="sm", name=f"po_{do}_{ch}_{s2}"
                )
            cs = slice(ch * 1024 + s2 * 512, ch * 1024 + s2 * 512 + 512)
            for hp in range(2):
                nc.tensor.matmul(
                    po,
                    lhsT=wo_sb[:, hp, do * 128 : (do + 1) * 128],
                    rhs=av_fm[:, hp, cs],
                    start=(hp == 0),
                    stop=(hp == 1),
                )
            if s2 == 0:
                ob_tiles[(do, ch)] = work.tile(
                    [128, 1024], bf16, tag="ob", name=f"ob_{do}_{ch}"
                )
            ob = ob_tiles[(do, ch)]
            if (do + s2) % 2 == 0:
                nc.scalar.copy(out=ob[:, s2 * 512 : (s2 + 1) * 512], in_=po)
            else:
                nc.vector.tensor_copy(
                    out=ob[:, s2 * 512 : (s2 + 1) * 512], in_=po
                )
            if s2 == 1:
                nc.sync.dma_start(
                    out_r[:, do, ch * 1024 : (ch + 1) * 1024], ob
                )

        # h1-qh0 runs second: it needs only the half-0 rope tables (ready
        # early), filling the Act gap while the half-1 rope chain still
        # drains on the DVE; h0-qh1 follows once those tables exist. Filler
        # deadlines are positional and the pre-h2 iteration count is
        # unchanged, so the pop schedule is identical.
        for h, qh in ((0, 0), (1, 0), (0, 1), (1, 1), (2, 0), (2, 1), (3, 0)):
            attention(h, qh)
        # out-projection for query half 0 rides the h3-qh1 filler slots (all
        # heads' qh0 av_fm is written once (3,0) finalizes at sweep entry)
        for do in range(8):
            for s2 in range(2):
                fillers.append(lambda d=do, s=s2: outproj_chunk(d, 0, s))
        attention(3, 1)
        while fillers:
            fillers.pop(0)()
        while pend:
            pend_pop()
        for do in range(8):
            for s2 in range(2):
                outproj_chunk(do, 1, s2, spread=True)
        attn_ps.close()
        if debug:
            nc.sync.dma_start(d_rbc[:, :], r_bc)
            nc.sync.dma_start(d_qk.rearrange("p (f t) -> p f t", f=4), qk_sb)
            nc.sync.dma_start(d_qk2.rearrange("p (f t) -> p f t", f=4), qk2_sb)
            nc.sync.dma_start(
                d_v.rearrange("p (a b c) -> p a b c", a=16, b=HPC), v_sb
            )
            nc.sync.dma_start(
                d_avtok.rearrange("p (a b c) -> p a b c", a=HPC, b=16), av_tok
            )
            nc.sync.dma_start(d_avfm.rearrange("p (a t) -> p a t", a=2), av_fm)
    nc.compile()
    return nc


def _host_inputs(x, norm_w, w_qkv, w_o, sin, cos):
    """Build the 8 per-core input maps (all bf16)."""
    n = T
    w_eff = np.asarray(w_qkv, np.float64) * np.asarray(norm_w, np.float64)[:, None]
    sin_n = np.asarray(sin, np.float32)[:n]  # [T, 64]
    cos_n = np.asarray(cos, np.float32)[:n]
    sign = np.concatenate([-np.ones(32, np.float32), np.ones(32, np.float32)])
    cos_tile = np.tile(cos_n.T, (2, 1))  # [128, T]
    sin_tile = np.tile((sin_n * sign[None, :]).T, (2, 1))  # [128, T]
    ql = np.arange(128)[None, :]
    key = np.arange(128)[:, None]
    masks = (ql >= key).astype(np.float32)
    ident_np = np.eye(128, dtype=np.float32)

    in_maps = []
    for c in range(8):
        b, g = c // 4, c % 4
        fs = slice(g * F, (g + 1) * F)
        in_maps.append(
            {
                "xT": np.ascontiguousarray(np.asarray(x, np.float32)[b].T).astype(BF16),
                "wq": (w_eff[:, 0:DIM][:, fs] * (DIM_HEAD ** -0.5)).astype(BF16),
                "wk": w_eff[:, DIM : 2 * DIM][:, fs].astype(BF16),
                "wv": w_eff[:, 2 * DIM : 3 * DIM][:, fs].astype(BF16),
                "wo": np.asarray(w_o, np.float32)[fs, :].astype(BF16),
                "cosT": cos_tile.astype(BF16),
                "sinT": sin_tile.astype(BF16),
                "masks": masks.astype(BF16),
                "ident": ident_np.astype(BF16),
            }
        )
    return in_maps


def kernel(x, norm_w, w_qkv, w_o, b_o, sin, cos):
    from concourse.bass_utils import run_bass_kernel_spmd

    if "nc" not in _NC_CACHE:
        _NC_CACHE["nc"] = _build_nc()
    nc = _NC_CACHE["nc"]
    in_maps = _host_inputs(x, norm_w, w_qkv, w_o, sin, cos)
    trace = bool(int(os.environ.get("KERNEL_TRACE", "0")))
    res = run_bass_kernel_spmd(nc, in_maps, core_ids=list(range(8)), trace=trace)
    if trace and res.exec_time_ns is not None:
        print(f"HW exec time: {res.exec_time_ns} ns")
    outs = [r["out"].astype(np.float32) for r in res.results]  # [1024, T] fm
    b_o = np.asarray(b_o, np.float32)
    full = np.empty((B, T, DIM), np.float32)
    for b in range(B):
        acc = outs[b * 4] + outs[b * 4 + 1] + outs[b * 4 + 2] + outs[b * 4 + 3]
        full[b] = acc.T + b_o[None, :]
    return full


# revision 78
# speedup vs baseline: 1.0168x; 1.0033x over previous
"""Trainium2 8-core kernel for RMSNorm -> QKV -> RoPE -> causal SDPA -> out-proj.

Sharding: core c = b*4 + g handles batch b (of 2) and heads 4g..4g+3 (of 16).
Each core computes a partial out-projection [dim, tokens]; the host sums the
4 head-group partials per batch and adds b_o.

Key layout/scheduling choices (cost model: matmul time = out-free-size rows,
vector/scalar op time = max free size):
- RMSNorm r is folded into BOTH Q and K rope tables (cosr/sinr = cos/sin * r),
  so softmax exp needs no per-key scale; V gets r via a token-major
  tensor_scalar. r itself comes from a DVE square + kc-tree-sum + gpsimd
  partition_all_reduce (no PE time), plus 16 one-row ones-matmuls for the
  token-major r_tok.
- rotate-half is 4 partition-permuting SBUF->SBUF DMAs per projection block
  (no PE perm matmul, no PSUM).
- Scores are computed key-major [128 keys, q] with exact causal starts.
- AV is computed TRANSPOSED: lhsT = exp tile [128 keys, 128 q], rhs = V
  [128 keys, 65] -> avt [q, d] at 65 rows/matmul (the PE streams only the
  65-wide rhs). Column 64 accumulates the softmax denominator (V's 65th
  column is ones); normalization is a per-partition reciprocal + one
  stride-0-broadcast multiply. Each q-tile's key-block accumulation is
  emitted as CONSECUTIVE matmuls: on real HW a PSUM bank supports only one
  open accumulation group at a time (interleaved-region accumulation that
  the interpreter accepts silently corrupts on silicon).
- av (token-major) -> feature-major for the out-projection via PE identity
  transposes (the XBAR dma_start_transpose's blocked-3D mode does not match
  interpreter semantics on HW).
- V projection and the second half of the QKV projections are emitted as
  "filler" work inside the attention kb loops so the PE stays busy while the
  Activation engine (the attention-era bottleneck) streams exp; AV groups
  lag the exp stream by one key block and sweep finalization rides the same
  deferred queue past head boundaries.
"""

import os

import numpy as np
import ml_dtypes

BF16 = ml_dtypes.bfloat16

DIM = 1024
HEADS = 16
DIM_HEAD = 64
T = 2048  # tokens per batch
B = 2
HPC = 4  # heads per core
F = HPC * DIM_HEAD  # 256 per-core head width
KC = DIM // 128  # 8 contraction chunks

_NC_CACHE = {}


def _build_nc():
    import concourse.bacc as bacc
    import concourse.mybir as mybir
    import concourse.tile as tile
    from concourse import bass_isa
    from contextlib import ExitStack

    f32 = mybir.dt.float32
    bf16 = mybir.dt.bfloat16
    nc = bacc.Bacc()

    xT = nc.declare_dram_parameter("xT", [DIM, T], bf16, isOutput=False)
    wq = nc.declare_dram_parameter("wq", [DIM, F], bf16, isOutput=False)
    wk = nc.declare_dram_parameter("wk", [DIM, F], bf16, isOutput=False)
    wv = nc.declare_dram_parameter("wv", [DIM, F], bf16, isOutput=False)
    wo = nc.declare_dram_parameter("wo", [F, DIM], bf16, isOutput=False)
    cosT = nc.declare_dram_parameter("cosT", [128, T], bf16, isOutput=False)
    sinT = nc.declare_dram_parameter("sinT", [128, T], bf16, isOutput=False)
    masks = nc.declare_dram_parameter("masks", [128, 128], bf16, isOutput=False)
    ident = nc.declare_dram_parameter("ident", [128, 128], bf16, isOutput=False)
    out = nc.declare_dram_parameter("out", [DIM, T], bf16, isOutput=True)
    debug = bool(int(os.environ.get("KERNEL_DEBUG", "0")))
    if debug:
        d_rbc = nc.declare_dram_parameter("d_rbc", [128, T], mybir_dt_f32 := __import__("concourse.mybir", fromlist=["dt"]).dt.float32, isOutput=True)
        d_qk = nc.declare_dram_parameter("d_qk", [128, 4 * T], bf16, isOutput=True)
        d_qk2 = nc.declare_dram_parameter("d_qk2", [128, 4 * T], bf16, isOutput=True)
        d_v = nc.declare_dram_parameter("d_v", [128, 16 * HPC * 65], bf16, isOutput=True)
        d_avtok = nc.declare_dram_parameter("d_avtok", [128, HPC * 16 * 64], bf16, isOutput=True)
        d_avfm = nc.declare_dram_parameter("d_avfm", [128, 2 * T], bf16, isOutput=True)

    Exp = mybir.ActivationFunctionType.Exp
    Sqrt = mybir.ActivationFunctionType.Sqrt
    mult = mybir.AluOpType.mult
    add = mybir.AluOpType.add

    nc_ = nc

    def stt_mul(out, a, b):
        return nc_.vector.tensor_mul(out, a, b)

    def stt_add(out, a, b):
        return nc_.vector.tensor_tensor(out, a, b, add)

    with ExitStack() as ctx:
        tc = ctx.enter_context(tile.TileContext(nc))
        consts = ctx.enter_context(tc.tile_pool(name="consts", bufs=1))
        persist = ctx.enter_context(tc.tile_pool(name="persist", bufs=1))
        work = ctx.enter_context(tc.tile_pool(name="work", bufs=2))
        vecs = ctx.enter_context(tc.tile_pool(name="vecs", bufs=2))

        # ---- constants / inputs ----
        wq_sb = consts.tile([128, KC, F], bf16, tag="wq")
        wk_sb = consts.tile([128, KC, F], bf16, tag="wk")
        wv_sb = consts.tile([128, KC, F], bf16, tag="wv")
        wo_sb = consts.tile([128, 2, DIM], bf16, tag="wo")
        cos_sb = consts.tile([128, T], bf16, tag="cos")
        sin_sb = consts.tile([128, T], bf16, tag="sin")
        mask_sb = consts.tile([128, 128], bf16, tag="mask")
        ones_col = consts.tile([128, 1], bf16, tag="onesc")
        id_sb = consts.tile([128, 128], bf16, tag="ident")

        xT_sb = persist.tile([128, KC, T], bf16, tag="xT")
        qk_sb = persist.tile([128, 4, T], bf16, tag="qk")  # raw projections
        qk2_sb = persist.tile([128, 4, T], bf16, tag="qk2")  # roped projections
        v_sb = persist.tile([128, 16, HPC, 65], bf16, tag="v")
        cosr_sb = persist.tile([128, T], bf16, tag="cosr")
        sinr_sb = persist.tile([128, T], bf16, tag="sinr")
        sq_tok = persist.tile([128, 16], f32, tag="sqtok")
        r_tok = persist.tile([128, 16], f32, tag="rtok")
        av_tok = persist.tile([128, HPC, 16, DIM_HEAD], bf16, tag="avtok")
        av_fm = persist.tile([128, 2, T], bf16, tag="avfm")

        xT_r = xT.rearrange("(kc p) t -> p kc t", p=128)
        nc.sync.dma_start(wk_sb, wk.rearrange("(kc p) f -> p kc f", p=128))
        for kc in range(KC):
            nc.sync.dma_start(xT_sb[:, kc], xT_r[:, kc])
        nc.sync.dma_start(wq_sb, wq.rearrange("(kc p) f -> p kc f", p=128))
        nc.sync.dma_start(wv_sb, wv.rearrange("(kc p) f -> p kc f", p=128))
        nc.sync.dma_start(cos_sb, cosT[:, :])
        nc.sync.dma_start(sin_sb, sinT[:, :])
        nc.sync.dma_start(mask_sb, masks[:, :])
        nc.sync.dma_start(id_sb, ident[:, :])
        nc.sync.dma_start(wo_sb, wo.rearrange("(fc p) d -> p fc d", p=128))
        nc.vector.memset(ones_col, 1.0)
        nc.vector.memset(v_sb[:, :, :, 64:65], 1.0)

        # ---- PSUM pools for the main body ----
        attn_ps = ExitStack()
        # "big" 2-bank slots serve the wide QK-proj groups, wide score tiles
        # and ss_tok; "sm" 1-bank slots serve narrow score tiles, filler
        # V-projections and filler QK-proj slices. Every use is transient.
        # avt: two single-buffered 65-wide accumulators per (h, qh); column 64
        # accumulates the softmax denominator (V's 65th column is ones).
        psBig = attn_ps.enter_context(tc.tile_pool(name="psBig", bufs=2, space="PSUM"))
        psSm = attn_ps.enter_context(tc.tile_pool(name="psSm", bufs=2, space="PSUM"))
        psAvt = attn_ps.enter_context(tc.tile_pool(name="psAvt", bufs=1, space="PSUM"))

        # ---- RMSNorm r (eps dropped: mean-square is O(1), below bf16 noise) ----
        # DVE: x^2 in kc-pair chunks (pipelines behind the xT chunk DMAs),
        # then an in-place kc tree-sum; the partition reduce + sqrt + recip +
        # cos/sin folds run split by column halves so the first half of the
        # rope tables is ready ~4us earlier.
        xsq_ctx = ExitStack()
        xsqp = xsq_ctx.enter_context(tc.tile_pool(name="xsqp", bufs=1))
        xsq = xsqp.tile([128, KC, T], bf16, tag="xsq")
        ss_bc = xsqp.tile([128, T], f32, tag="ssbc")
        sq_bc = xsqp.tile([128, T], bf16, tag="sqbc")
        r_bc = xsqp.tile([128, T], f32, tag="rbc")
        r_bcb = xsqp.tile([128, T], bf16, tag="rbcb")
        # running accumulator: each add consumes the just-arrived chunk pair,
        # so only [square, add, fold] trail the LAST xT chunk's DMA (the
        # critical path), instead of a full reduction tree
        for i in range(4):
            stt_mul(
                xsq[:, 2 * i : 2 * i + 2],
                xT_sb[:, 2 * i : 2 * i + 2],
                xT_sb[:, 2 * i : 2 * i + 2],
            )
            if i > 0:
                stt_add(
                    xsq[:, 0:2],
                    xsq[:, 0:2],
                    xsq[:, 2 * i : 2 * i + 2],
                )
        stt_add(xsq[:, 0], xsq[:, 0], xsq[:, 1])
        ssum = xsq[:, 0]

        def r_quarter(qt_):
            cs = slice(qt_ * 512, (qt_ + 1) * 512)
            nc.gpsimd.partition_all_reduce(
                ss_bc[:, cs], ssum[:, cs], 128, bass_isa.ReduceOp.add
            )
            nc.scalar.activation(sq_bc[:, cs], ss_bc[:, cs], Sqrt, scale=1.0 / DIM)
            nc.vector.reciprocal(r_bc[:, cs], sq_bc[:, cs])
            # bf16 copy of r (on the idle-early Act engine) lets the cos/sin
            # folds hit the DVE 4x mode
            nc.scalar.copy(out=r_bcb[:, cs], in_=r_bc[:, cs])
            stt_mul(cosr_sb[:, cs], cos_sb[:, cs], r_bcb[:, cs])
            stt_mul(sinr_sb[:, cs], sin_sb[:, cs], r_bcb[:, cs])

        def r_half(hf):
            r_quarter(2 * hf)
            r_quarter(2 * hf + 1)

        # ---- projection + rope helpers ----
        w_of = {0: (wq_sb, 0), 1: (wq_sb, 1), 2: (wk_sb, 0), 3: (wk_sb, 1)}

        def proj_cols(fidx, lo, width, copy_eng):
            """Project columns [lo, lo+width) of q/k block fidx into qk_sb."""
            wsb, fc = w_of[fidx]
            pool, tag = (psBig, "big") if width > 512 else (psSm, "sm")
            ps = pool.tile([128, width], f32, tag=tag, name=f"qkp_{fidx}_{lo}")
            for kc in range(KC):
                for s in range(0, width, 512):
                    w = min(512, width - s)
                    nc.tensor.matmul(
                        ps[:, s : s + w],
                        lhsT=wsb[:, kc, fc * 128 : (fc + 1) * 128],
                        rhs=xT_sb[:, kc, lo + s : lo + s + w],
                        start=(kc == 0),
                        stop=(kc == KC - 1),
                    )
            if copy_eng == "act":
                nc.scalar.copy(out=qk_sb[:, fidx, lo : lo + width], in_=ps)
            else:
                nc.vector.tensor_copy(out=qk_sb[:, fidx, lo : lo + width], in_=ps)

        def rope_cols(fidx, lo, width):
            """qk2[fidx] = qk[fidx]*cosr + rotate_half(qk[fidx])*sinr."""
            cs = slice(lo, lo + width)
            pt = work.tile([128, width], bf16, tag="perm", name=f"pm_{fidx}_{lo}")
            src = qk_sb[:, fidx, cs]
            for d0, s0 in ((0, 32), (32, 0), (64, 96), (96, 64)):
                nc.sync.dma_start(pt[d0 : d0 + 32], src[s0 : s0 + 32])
            t2 = work.tile([128, width], bf16, tag="t2", name=f"t2_{fidx}_{lo}")
            stt_mul(t2, qk_sb[:, fidx, cs], cosr_sb[:, cs])
            t1 = work.tile([128, width], bf16, tag="t1", name=f"t1_{fidx}_{lo}")
            stt_mul(t1, pt, sinr_sb[:, cs])
            stt_add(qk2_sb[:, fidx, cs], t2, t1)

        def v_group(tt, hp):
            """Project V for token block tt, head pair hp, and apply r."""
            psv = psSm.tile([128, 128], f32, tag="sm", name=f"psv_{tt}_{hp}")
            for kc in range(KC):
                nc.tensor.matmul(
                    psv,
                    lhsT=xT_sb[:, kc, tt * 128 : (tt + 1) * 128],
                    rhs=wv_sb[:, kc, hp * 128 : (hp + 1) * 128],
                    start=(kc == 0),
                    stop=(kc == KC - 1),
                )
            nc.vector.tensor_scalar_mul(
                v_sb[:, tt, 2 * hp : 2 * hp + 2, 0:64],
                psv.rearrange("p (h d) -> p h d", h=2),
                r_tok[:, tt : tt + 1],
            )

        # Q/K projections for heads 0,1 (fidx 2=k, 0=q). PE starts on these as
        # xT chunks arrive; Act does the PSUM->SBUF copies (it is idle until
        # the first exp); DVE owns the RMS chain so it never blocks a copy.
        for fidx in (2, 0):
            proj_cols(fidx, 0, 1024, "act")
            proj_cols(fidx, 1024, 1024, "act")
        # r + rope, half 0 first so h0-qh0 attention can start early
        r_half(0)
        rope_cols(2, 0, 1024)
        rope_cols(0, 0, 1024)
        r_half(1)
        if debug:
            nc.sync.dma_start(d_rbc[:, :], r_bc)
        xsq_ctx.close()
        expp = ctx.enter_context(tc.tile_pool(name="expp", bufs=17))
        rope_cols(2, 1024, 1024)
        rope_cols(0, 1024, 1024)
        # bridge the PE gap between the projections and the first score matmul
        v_group(0, 0)
        v_group(1, 0)
        v_group(2, 0)
        v_group(3, 0)
        # token-major r (for V): per-token sums via 16 one-row ones-matmuls.
        # Emitted after the V pre-groups: they wait on the full xsq tree, and
        # with a 4-deep PE wait queue they would head-of-line-block the ready
        # V projection matmuls behind them.
        ss_tok = psBig.tile([128, 16], f32, tag="big", name="ss_tok")
        for tt in range(16):
            nc.tensor.matmul(
                ss_tok[:, tt : tt + 1],
                lhsT=ssum[:, tt * 128 : (tt + 1) * 128],
                rhs=ones_col,
                start=True,
                stop=True,
            )
        nc.scalar.activation(sq_tok, ss_tok, Sqrt, scale=1.0 / DIM)
        nc.vector.reciprocal(r_tok, sq_tok)

        # fillers: one popped per attention kb iteration, sized ~0.4us each so
        # they never delay the next score matmul by more than one exp. V-hp0
        # (heads 0,1) front-runs the h0/h1 kb sweeps; the fidx 3/1 projections
        # and ropes complete during h1; V-hp1 front-runs h2 (lag-4 covers the
        # small pop-vs-use slack).
        fillers = [(lambda tt=tt: v_group(tt, 0)) for tt in range(4, 16)]
        for fidx in (3, 1):
            for e in range(8):
                fillers.append(
                    lambda f=fidx, e=e: proj_cols(f, e * 128, 128, "dve")
                )
            fillers.append(lambda f=fidx: rope_cols(f, 0, 1024))
            for e in range(8, 16):
                fillers.append(
                    lambda f=fidx, e=e: proj_cols(f, e * 128, 128, "dve")
                )
            if fidx == 1:
                # slip the first two V-hp1 groups ahead of the final rope
                # (which has 8 iterations of deadline slack into h2) so the
                # h2 AV matmuls get 2 iterations of V-scale slack
                fillers.append(lambda: v_group(0, 1))
                fillers.append(lambda: v_group(1, 1))
            fillers.append(lambda f=fidx: rope_cols(f, 1024, 1024))
        fillers += [(lambda tt=tt: v_group(tt, 1)) for tt in range(2, 16)]

        # ---- attention ----
        # pend holds (entry, fin), one entry per query tile jt. Each entry
        # emits the FULL kb-accumulation for that q-tile as consecutive
        # matmuls — PSUM accumulation groups must be contiguous per bank on
        # real HW. Entries lag the exp stream by one key block so they don't
        # wait on an exp semaphore, and sweep finalization (normalize +
        # PE transpose to feature-major) rides the same queue past sweep
        # boundaries so head transitions never stall the PE.
        pend = []

        def avt_mms(h, qh, jt, exs, avts):
            qlo = qh * 1024
            jl = jt - qh * 8
            for kb in range(jt + 1):
                ex, c0 = exs[kb]
                off = jt * 128 - qlo - c0
                nc.tensor.matmul(
                    avts[jl // 4][:, jl % 4, :],
                    lhsT=ex[:, off : off + 128],
                    rhs=v_sb[:, kb, h],
                    start=(kb == 0),
                    stop=(kb == jt),
                )

        def pend_pop():
            entry, fin = pend.pop(0)
            avt_mms(*entry)
            if fin is not None:
                fin()

        def make_finalize(h, qh, avts):
            def fin():
                for half in range(2):
                    rec = vecs.tile(
                        [128, 4],
                        f32,
                        tag=f"rec{half}",
                        name=f"rec_{h}_{qh}_{half}",
                    )
                    nc.vector.reciprocal(rec, avts[half][:, :, 64])
                    nc.vector.tensor_tensor(
                        av_tok[:, h, qh * 8 + 4 * half : qh * 8 + 4 * half + 4, :],
                        avts[half][:, :, 0:64],
                        rec.broadcast_to([128, 4, DIM_HEAD]),
                        mult,
                    )
                # token-major -> feature-major via PE identity transposes
                r0 = (h % 2) * 64
                for half in range(2):
                    ptr = psSm.tile(
                        [64, 4, 128], bf16, tag="sm", name=f"tr_{h}_{qh}_{half}"
                    )
                    for j in range(4):
                        nc.tensor.transpose(
                            ptr[:, j, :],
                            av_tok[:, h, qh * 8 + 4 * half + j, :],
                            id_sb,
                        )
                    nc.vector.tensor_copy(
                        out=av_fm[
                            r0 : r0 + 64,
                            h // 2,
                            qh * 1024 + half * 512 : qh * 1024 + half * 512 + 512,
                        ],
                        in_=ptr,
                    )

            return fin

        def attention(h, qh):
            qt = qk2_sb[:, 0 if h < 2 else 1]
            kt = qk2_sb[:, 2 if h < 2 else 3]
            rows = slice((h % 2) * 64, (h % 2) * 64 + 64)
            qlo = qh * 1024
            nkb = 8 * (qh + 1)

            def sc_exp(kb, mid=None):
                """Emit the score matmuls + exp (+ mask) for key block kb.
                `mid` (the pend pops) runs between the matmuls and the exp
                tile allocation — popped entries read old exp tiles whose
                pool slots the new tile reuses."""
                c0 = max(kb * 128 - qlo, 0)
                W = 1024 - c0
                if W > 512:
                    sc = psBig.tile(
                        [128, 1024], f32, tag="big", name=f"sc_{h}_{qh}_{kb}"
                    )
                else:
                    sc = psSm.tile(
                        [128, 512], f32, tag="sm", name=f"sc_{h}_{qh}_{kb}"
                    )
                for o in range(0, W, 512):
                    w = min(512, W - o)
                    nc.tensor.matmul(
                        sc[:, o : o + w],
                        lhsT=kt[rows, kb * 128 : (kb + 1) * 128],
                        rhs=qt[rows, qlo + c0 + o : qlo + c0 + o + w],
                        start=True,
                        stop=True,
                    )
                if mid is not None:
                    mid()
                ex = expp.tile(
                    [128, 1024], bf16, tag="exp", name=f"ex_{h}_{qh}_{kb}"
                )
                nc.scalar.activation(ex[:, 0:W], sc[:, 0:W], Exp)
                if kb * 128 >= qlo:
                    stt_mul(ex[:, 0:128], ex[:, 0:128], mask_sb)
                return ex, c0

            def pops():
                while len(pend) > 1:
                    pend_pop()

            # drain the previous sweep (its last q-tile group + finalize)
            # before its exp tiles' pool slots get reused below
            while pend:
                pend_pop()
            avts = (
                psAvt.tile([128, 4, 65], f32, tag="avt_a", name=f"avta_{h}_{qh}"),
                psAvt.tile([128, 4, 65], f32, tag="avt_b", name=f"avtb_{h}_{qh}"),
            )
            exs = [sc_exp(0)]
            if fillers:
                fillers.pop(0)()
            if qh == 0:
                pend.append(((h, qh, 0, exs, avts), None))
            for kb in range(1, nkb):
                exs.append(sc_exp(kb, mid=pops))
                if fillers:
                    fillers.pop(0)()
                if kb >= qh * 8:
                    jt = kb
                    fin = make_finalize(h, qh, avts) if kb == nkb - 1 else None
                    pend.append(((h, qh, jt, exs, avts), fin))

        # ---- out projection chunks (partial over heads; host sums) ----
        out_r = out.rearrange("(do p) t -> p do t", p=128)
        ob_tiles = {}

        def outproj_chunk(do, ch, s2, spread=False):
            """512 output columns for output-row block do, column half ch.
            spread=True (tail only, when attention no longer needs PSUM)
            alternates chunks across the sm and big pools so the rotation is
            two output-blocks deep instead of lockstepping on one slot."""
            if spread and (2 * do + s2) % 2 == 1:
                po = psBig.tile(
                    [128, 512], f32, tag="big", name=f"po_{do}_{ch}_{s2}"
                )
            else:
                po = psSm.tile(
                    [128, 512], f32, tag="sm", name=f"po_{do}_{ch}_{s2}"
                )
            cs = slice(ch * 1024 + s2 * 512, ch * 1024 + s2 * 512 + 512)
            for hp in range(2):
                nc.tensor.matmul(
                    po,
                    lhsT=wo_sb[:, hp, do * 128 : (do + 1) * 128],
                    rhs=av_fm[:, hp, cs],
                    start=(hp == 0),
                    stop=(hp == 1),
                )
            if s2 == 0:
                ob_tiles[(do, ch)] = work.tile(
                    [128, 1024], bf16, tag="ob", name=f"ob_{do}_{ch}"
                )
            ob = ob_tiles[(do, ch)]
            if (do + s2) % 2 == 0:
                nc.scalar.copy(out=ob[:, s2 * 512 : (s2 + 1) * 512], in_=po)
            else:
                nc.vector.tensor_copy(
                    out=ob[:, s2 * 512 : (s2 + 1) * 512], in_=po
                )
            if s2 == 1:
                nc.sync.dma_start(
                    out_r[:, do, ch * 1024 : (ch + 1) * 1024], ob
                )

        # h1-qh0 runs second: it needs only the half-0 rope tables (ready
        # early), filling the Act gap while the half-1 rope chain still
        # drains on the DVE; h0-qh1 follows once those tables exist. Filler
        # deadlines are positional and the pre-h2 iteration count is
        # unchanged, so the pop schedule is identical.
        for h, qh in ((0, 0), (1, 0), (0, 1), (1, 1), (2, 0), (2, 1), (3, 0)):
            attention(h, qh)
        # out-projection for query half 0 rides the h3-qh1 filler slots (all
        # heads' qh0 av_fm is written once (3,0) finalizes at sweep entry)
        for do in range(8):
            for s2 in range(2):
                fillers.append(lambda d=do, s=s2: outproj_chunk(d, 0, s))
        attention(3, 1)
        while fillers:
            fillers.pop(0)()
        while pend:
            pend_pop()
        for do in range(8):
            for s2 in range(2):
                outproj_chunk(do, 1, s2, spread=True)
        attn_ps.close()
        if debug:
            nc.sync.dma_start(d_rbc[:, :], r_bc)
            nc.sync.dma_start(d_qk.rearrange("p (f t) -> p f t", f=4), qk_sb)
            nc.sync.dma_start(d_qk2.rearrange("p (f t) -> p f t", f=4), qk2_sb)
            nc.sync.dma_start(
                d_v.rearrange("p (a b c) -> p a b c", a=16, b=HPC), v_sb
            )
            nc.sync.dma_start(
                d_avtok.rearrange("p (a b c) -> p a b c", a=HPC, b=16), av_tok
            )
            nc.sync.dma_start(d_avfm.rearrange("p (a t) -> p a t", a=2), av_fm)
    nc.compile()
    return nc


def _host_inputs(x, norm_w, w_qkv, w_o, sin, cos):
    """Build the 8 per-core input maps (all bf16)."""
    n = T
    w_eff = np.asarray(w_qkv, np.float64) * np.asarray(norm_w, np.float64)[:, None]
    sin_n = np.asarray(sin, np.float32)[:n]  # [T, 64]
    cos_n = np.asarray(cos, np.float32)[:n]
    sign = np.concatenate([-np.ones(32, np.float32), np.ones(32, np.float32)])
    cos_tile = np.tile(cos_n.T, (2, 1))  # [128, T]
    sin_tile = np.tile((sin_n * sign[None, :]).T, (2, 1))  # [128, T]
    ql = np.arange(128)[None, :]
    key = np.arange(128)[:, None]
    masks = (ql >= key).astype(np.float32)
    ident_np = np.eye(128, dtype=np.float32)

    in_maps = []
    for c in range(8):
        b, g = c // 4, c % 4
        fs = slice(g * F, (g + 1) * F)
        in_maps.append(
            {
                "xT": np.ascontiguousarray(np.asarray(x, np.float32)[b].T).astype(BF16),
                "wq": (w_eff[:, 0:DIM][:, fs] * (DIM_HEAD ** -0.5)).astype(BF16),
                "wk": w_eff[:, DIM : 2 * DIM][:, fs].astype(BF16),
                "wv": w_eff[:, 2 * DIM : 3 * DIM][:, fs].astype(BF16),
                "wo": np.asarray(w_o, np.float32)[fs, :].astype(BF16),
                "cosT": cos_tile.astype(BF16),
                "sinT": sin_tile.astype(BF16),
                "masks": masks.astype(BF16),
                "ident": ident_np.astype(BF16),
            }
        )
    return in_maps


def kernel(x, norm_w, w_qkv, w_o, b_o, sin, cos):
    from concourse.bass_utils import run_bass_kernel_spmd

    if "nc" not in _NC_CACHE:
        _NC_CACHE["nc"] = _build_nc()
    nc = _NC_CACHE["nc"]
    in_maps = _host_inputs(x, norm_w, w_qkv, w_o, sin, cos)
    trace = bool(int(os.environ.get("KERNEL_TRACE", "0")))
    res = run_bass_kernel_spmd(nc, in_maps, core_ids=list(range(8)), trace=trace)
    if trace and res.exec_time_ns is not None:
        print(f"HW exec time: {res.exec_time_ns} ns")
    outs = [r["out"].astype(np.float32) for r in res.results]  # [1024, T] fm
    b_o = np.asarray(b_o, np.float32)
    full = np.empty((B, T, DIM), np.float32)
    for b in range(B):
        acc = outs[b * 4] + outs[b * 4 + 1] + outs[b * 4 + 2] + outs[b * 4 + 3]
        full[b] = acc.T + b_o[None, :]
    return full
